# revision 22
# baseline (speedup 1.0000x reference)
"""C2Q attention kernel for 8 TRN2 NeuronCores.

Math (per batch):
    u      = (o_q @ W.T) / sqrt(H)               [Tq, H]
    score  = o_c @ u.T                           [Tc, Tq]
    prob   = softmax_j(score masked at j>=q_len) [Tc, Tq]
    out    = (prob * (i < c_len)) @ o_q          [Tc, H]

The Linear bias is dropped: it adds sum_h o_c[i,h]*b[h] to score[i, j] for
every j, i.e. a per-i constant, which softmax over j cancels exactly; the
context matmul uses raw o_q, so b never reaches the output.

Device layout choices (everything lands K-on-partitions with zero on-chip
transposes of activations):
    u computed as [o, j]  (lhsT = W.T[h, o] tile, rhs = o_qT[h, j])
    score computed TRANSPOSED e=[j, i] (lhsT = u[o, j-block], rhs = o_cT[o, i])
    exp via ACT with per-partition bias qb[j] in {0, -6e4}: masked -> exactly 0
    denominator d[1, i] = ones[j,1].T @ e  (matmul partition-reduce)
    1/d transposed to columns via K=1 matmuls, folded into context eviction
    context [i, h] = e[j, i-block].T @ o_q[j, h]   (natural output layout)
c_len row masking is applied host-side (those rows are zeroed, never read).

Ragged specialization: the program is built AFTER the inputs are known, so
per-batch tile counts jt=ceil(q_len/128), it=ceil(c_len/128) are baked in.
Batches are assigned to (core, slot) pairs; SPMD requires every core to run
the same program, so slot s uses the max (J_s, I_s) over the 8 batches
assigned to it; the assignment minimizes total padded work. Masked-out j
tiles contribute e == 0 exactly (exp bias), so skipping them is lossless;
rows i in [c_len, I*128) are computed but never read by the host gather.

IMPORTANT (measured, not theorized): DRAM parameter sizes and DMA transfer
shapes must be IDENTICAL across slots. Builds with per-slot ragged slab
widths run the whole NeuronCore ~20% slower (2.0 vs 2.4 GHz effective on
every engine, from t=0). So all slabs/DMAs are full-size; only the compute
(matmul trip counts / AP sizes) is specialized.

Slot 0 is special-cased for the DMA ramp: its Linear runs k-major with all
8 PSUM banks as accumulators (the PE has 8 runnable matmuls the moment each
(wt_k, oqT_k) pair lands), and its score runs o-outer with J concurrent
PSUM groups (each o step needs only ocT_k[o], so the PE chases the ocT DMA
tail instead of stalling on the full slab set; a PE stall here also drops
the PE to a low pstate which costs ~2us extra to ramp back).
"""

import os
import sys

import numpy as np

if "/opt/trn_rl_repo" not in sys.path:
    sys.path.insert(0, "/opt/trn_rl_repo")

B, Tc, Tq, H = 32, 512, 512, 1024
N_CORES = 8
N_SLOTS = B // N_CORES  # 4 batches per core, one per slot
KT = H // 128  # contraction tiles over h (8)
OT = H // 128  # linear-output tiles over o (8)
JT = Tq // 128
IT = Tc // 128
HB = H // 512  # free-dim blocks for context matmul (2)
SCALE = 1.0 / 32.0  # 1/sqrt(H)
QTW = Tq + 8  # uniform slab widths (see docstring)
CTW = Tc + 8


def _slot_cost(j, i):
    # PE rows: Linear 64*j*128, score j*8*i*128, d i*128, ctx i*2*j*512
    return 8192 * j + 2048 * j * i + 128 * i


def _assign_slots(jt, it):
    """Partition the B batches into N_SLOTS groups of N_CORES, minimizing
    sum over groups of _slot_cost(maxJ, maxI). Returns list of groups
    (each a list of batch indices), sorted largest-cost first."""
    idx = list(range(B))
    best = None
    for key in (
        lambda b: (jt[b], it[b]),
        lambda b: (it[b], jt[b]),
        lambda b: _slot_cost(jt[b], it[b]),
    ):
        order = sorted(idx, key=key, reverse=True)
        slots = [order[s * N_CORES : (s + 1) * N_CORES] for s in range(N_SLOTS)]

        def tot(slots):
            return sum(
                _slot_cost(max(jt[b] for b in g), max(it[b] for b in g))
                for g in slots
            )

        improved = True
        while improved:
            improved = False
            for s1 in range(N_SLOTS):
                for s2 in range(s1 + 1, N_SLOTS):
                    for a in range(N_CORES):
                        for b2 in range(N_CORES):
                            cur = tot(slots)
                            slots[s1][a], slots[s2][b2] = (
                                slots[s2][b2],
                                slots[s1][a],
                            )
                            if tot(slots) < cur:
                                improved = True
                            else:
                                slots[s1][a], slots[s2][b2] = (
                                    slots[s2][b2],
                                    slots[s1][a],
                                )
            if best is None or tot(slots) < best[0]:
                best = (tot(slots), [list(g) for g in slots])
    slots = best[1]
    slots.sort(
        key=lambda g: _slot_cost(max(jt[b] for b in g), max(it[b] for b in g)),
        reverse=True,
    )
    return slots


def _build_program(slot_shapes):
    """slot_shapes: list of (J, I) per slot; one batch per core per slot."""
    import concourse.bacc as bacc
    import concourse.mybir as mybir
    import concourse.tile as tile

    f32 = mybir.dt.float32
    f16 = mybir.dt.float16
    Copy = mybir.ActivationFunctionType.Copy

    nc = bacc.Bacc("TRN2", debug=False)

    oqT_d, ocT_d, oqN_d, out_d, d_d = [], [], [], [], []
    for s in range(len(slot_shapes)):
        oqT_d.append(
            nc.declare_dram_parameter(f"oqT{s}", [KT, 128, QTW], f16, isOutput=False)
        )
        ocT_d.append(
            nc.declare_dram_parameter(f"ocT{s}", [KT, 128, CTW], f16, isOutput=False)
        )
        oqN_d.append(
            nc.declare_dram_parameter(f"oqN{s}", [Tq, H], f16, isOutput=False)
        )
        out_d.append(
            nc.declare_dram_parameter(f"out{s}", [Tc, H], f16, isOutput=True)
        )
        d_d.append(
            nc.declare_dram_parameter(f"dvec{s}", [1, Tc], f32, isOutput=True)
        )
    wt_d = nc.declare_dram_parameter("wt", [KT, 128, H], f16, isOutput=False)

    with tile.TileContext(nc) as tc:
        with (
            tc.tile_pool(name="const", bufs=1) as cpool,
            tc.tile_pool(name="inp", bufs=2) as ipool,
            tc.tile_pool(name="work", bufs=1) as wpool,
            tc.tile_pool(name="outp", bufs=3) as opool,
            tc.tile_pool(name="ps_u", bufs=2, space="PSUM") as ps_u,
            tc.tile_pool(name="ps_s", bufs=2, space="PSUM") as ps_s,
            tc.tile_pool(name="ps_c", bufs=3, space="PSUM") as ps_c,
            tc.tile_pool(name="ps_d", bufs=1, space="PSUM") as ps_d,
        ):
            ones_s = cpool.tile([1, 1], f32)
            nc.vector.memset(ones_s, 1.0)

            # PE warm-up: a short dependency-free matmul chain right after
            # boot ramps the PE out of its low pstate before the first real
            # operands land (~11us in); results are discarded. The dummy
            # PSUM group (dmisc bank) is free until slot 0's k-major Linear
            # reaches its 8th accumulator (~13us).
            dummy = cpool.tile([128, 512], f16, tag="dummy", name="dummy")
            nc.vector.memset(dummy, 0.0)
            dps_w = ps_d.tile([128, 512], f32, tag="dmisc", name="warm")
            for w in range(12):
                nc.tensor.matmul(
                    dps_w,
                    dummy[:, :128],
                    dummy[:, :512],
                    start=True,
                    stop=True,
                    skip_group_check=True,
                )

            wt_k = [cpool.tile([128, H], f16, tag=f"wt{k}", name=f"wt{k}") for k in range(KT)]

            def u_evict(o, dst, src):
                # alternate engines so the evict chain is ~2x faster; the
                # score matmuls consume u[o] in order
                if o % 2 == 0:
                    nc.vector.tensor_copy(out=dst, in_=src)
                else:
                    nc.scalar.activation(out=dst, in_=src, func=Copy)

            for s, (J, I) in enumerate(slot_shapes):
                qc, ic = J * 128, I * 128
                oqT_k = [ipool.tile([128, QTW], f16, tag=f"oqT{k}", name=f"oqT{k}_{s}") for k in range(KT)]
                ocT_k = [ipool.tile([128, CTW], f16, tag=f"ocT{k}", name=f"ocT{k}_{s}") for k in range(KT)]
                oqN = ipool.tile([128, JT, H], f16, tag="oqN", name=f"oqN_{s}")
                qb = oqT_k[KT - 1][:, Tq : Tq + J]
                ones = ocT_k[0][:, Tc : Tc + 1]
                for k in range(KT):
                    if s == 0:
                        nc.sync.dma_start(out=wt_k[k], in_=wt_d[k])
                    nc.sync.dma_start(out=oqT_k[k], in_=oqT_d[s][k])
                for k in range(KT):
                    nc.sync.dma_start(out=ocT_k[k], in_=ocT_d[s][k])
                for j in range(JT):
                    nc.sync.dma_start(
                        out=oqN[:, j, :], in_=oqN_d[s][j * 128 : (j + 1) * 128, :]
                    )

                # ---- Linear: u[o, j] = W@o_q.T  (the 1/32 scale rides in
                # the Exp activation's scale argument) ----
                u = wpool.tile([128, OT, qc], f16, tag="u", name=f"u_{s}")
                e_tiles = []
                dps = ps_d.tile([1, Tc], f32, tag="dmisc", name=f"dps_{s}")
                if s == 0:
                    ups_o = [
                        ps_u.tile([128, qc], f32, tag="ups", name="ups_a"),
                        ps_u.tile([128, qc], f32, tag="ups", name="ups_b"),
                        ps_s.tile([128, qc], f32, tag="sps", name="ups_c"),
                        ps_s.tile([128, qc], f32, tag="sps", name="ups_d"),
                        ps_c.tile([128, qc], f32, tag="cps", name="ups_e"),
                        ps_c.tile([128, qc], f32, tag="cps", name="ups_f"),
                        ps_c.tile([128, qc], f32, tag="cps", name="ups_g"),
                        ps_d.tile([128, qc], f32, tag="dmisc", name="ups_h"),
                    ]
                    for k in range(KT):
                        for o in range(OT):
                            nc.tensor.matmul(
                                ups_o[o],
                                wt_k[k][:, o * 128 : (o + 1) * 128],
                                oqT_k[k][:, :qc],
                                start=(k == 0),
                                stop=(k == KT - 1),
                            )
                    for o in range(OT):
                        u_evict(o, u[:, o, :], ups_o[o])

                    # ---- score o-outer: each o step needs only ocT_k[o],
                    # so the PE chases the ocT DMA tail ----
                    sps_j = [
                        ps_s.tile([128, ic], f32, tag="sps", name="sps_j0"),
                        ps_s.tile([128, ic], f32, tag="sps", name="sps_j1"),
                        ps_u.tile([128, ic], f32, tag="ups", name="sps_j2"),
                        ps_u.tile([128, ic], f32, tag="ups", name="sps_j3"),
                    ][:J]
                    for o in range(OT):
                        for jt_ in range(J):
                            nc.tensor.matmul(
                                sps_j[jt_],
                                u[:, o, jt_ * 128 : (jt_ + 1) * 128],
                                ocT_k[o][:, :ic],
                                start=(o == 0),
                                stop=(o == OT - 1),
                            )
                    for jt_ in range(J):
                        e = wpool.tile([128, ic], f16, tag=f"e{jt_}", name=f"e{jt_}_{s}")
                        nc.scalar.activation(
                            out=e,
                            in_=sps_j[jt_],
                            func=mybir.ActivationFunctionType.Exp,
                            bias=qb[:, jt_ : jt_ + 1],
                            scale=SCALE,
                        )
                        e_tiles.append(e)
                        nc.tensor.matmul(
                            dps,
                            ones,
                            e,
                            start=(jt_ == 0),
                            stop=(jt_ == J - 1),
                            skip_group_check=True,
                        )
                else:
                    for o in range(OT):
                        ups = ps_u.tile([128, qc], f32, tag="ups")
                        for k in range(KT):
                            nc.tensor.matmul(
                                ups,
                                wt_k[k][:, o * 128 : (o + 1) * 128],
                                oqT_k[k][:, :qc],
                                start=(k == 0),
                                stop=(k == KT - 1),
                            )
                        u_evict(o, u[:, o, :], ups)

                    # ---- score jt-outer + exp, denominator accumulation
                    # interleaved one step behind so its chain latency hides
                    for jt_ in range(J):
                        sps = ps_s.tile([128, ic], f32, tag="sps")
                        for o in range(OT):
                            nc.tensor.matmul(
                                sps,
                                u[:, o, jt_ * 128 : (jt_ + 1) * 128],
                                ocT_k[o][:, :ic],
                                start=(o == 0),
                                stop=(o == OT - 1),
                            )
                        e = wpool.tile([128, ic], f16, tag=f"e{jt_}", name=f"e{jt_}_{s}")
                        nc.scalar.activation(
                            out=e,
                            in_=sps,
                            func=mybir.ActivationFunctionType.Exp,
                            bias=qb[:, jt_ : jt_ + 1],
                            scale=SCALE,
                        )
                        e_tiles.append(e)
                        if jt_ >= 1:
                            nc.tensor.matmul(
                                dps,
                                ones,
                                e_tiles[jt_ - 1],
                                start=(jt_ == 1),
                                stop=False,
                                skip_group_check=True,
                            )
                    nc.tensor.matmul(
                        dps,
                        ones,
                        e_tiles[J - 1],
                        start=(J == 1),
                        stop=True,
                        skip_group_check=True,
                    )

                osb_tiles = {}

                def ctx_group(itb, hb, J=J, s=s, e_tiles=e_tiles, oqN=oqN, osb_tiles=osb_tiles):
                    if itb not in osb_tiles:
                        osb_tiles[itb] = opool.tile(
                            [128, H], f16, tag="osb", name=f"osb{itb}_{s}"
                        )
                    cps = ps_c.tile([128, 512], f32, tag="cps", name=f"cps{itb}{hb}_{s}")
                    for jt_ in range(J):
                        nc.tensor.matmul(
                            cps,
                            e_tiles[jt_][:, itb * 128 : (itb + 1) * 128],
                            oqN[:, jt_, hb * 512 : (hb + 1) * 512],
                            start=(jt_ == 0),
                            stop=(jt_ == J - 1),
                        )
                    return cps

                def ctx_evict(itb, hb, cps, s=s, osb_tiles=osb_tiles, out=out_d[s]):
                    # unnormalized context; the host divides by d (shipped
                    # separately), so the evict is a plain copy and can
                    # alternate engines like u_evict does
                    osb = osb_tiles[itb]
                    if hb % 2 == 0:
                        nc.vector.tensor_copy(
                            out=osb[:, hb * 512 : (hb + 1) * 512], in_=cps
                        )
                    else:
                        nc.scalar.activation(
                            out=osb[:, hb * 512 : (hb + 1) * 512], in_=cps, func=Copy
                        )
                    nc.sync.dma_start(
                        out=out[
                            itb * 128 : (itb + 1) * 128, hb * 512 : (hb + 1) * 512
                        ],
                        in_=osb[:, hb * 512 : (hb + 1) * 512],
                    )

                # d ships to the host (the division happens there); the
                # copy+DMA drain while the first ctx groups run
                cps00 = ctx_group(0, 0)
                dsb = wpool.tile([1, Tc], f32, tag="dsb", name=f"dsb_{s}")
                nc.vector.tensor_copy(out=dsb, in_=dps)
                nc.sync.dma_start(out=d_d[s][:, :], in_=dsb)

                cps01 = ctx_group(0, 1)
                ctx_evict(0, 0, cps00)
                ctx_evict(0, 1, cps01)
                for itb in range(1, I):
                    for hb in range(HB):
                        cps = ctx_group(itb, hb)
                        ctx_evict(itb, hb, cps)

    nc.compile()
    return nc


def _host_inputs(o_c, o_q, W, q_lengths, slots):
    """Per-core input maps (host-side sharding + re-layout). All slabs are
    full-size (uniform DMA shapes across slots; see module docstring)."""
    NEG16 = np.float16(-60000.0)  # exp(x - 60000) == 0 exactly in fp32
    wt_host = np.ascontiguousarray(W.T.reshape(KT, 128, H).astype(np.float16))
    jidx = np.arange(JT)[None, :] * 128 + np.arange(128)[:, None]  # [128, JT]
    in_maps = [{"wt": wt_host} for _ in range(N_CORES)]
    for s, grp in enumerate(slots):
        for c, g in enumerate(grp):
            oqT = np.zeros((KT, 128, QTW), np.float16)
            oqT[:, :, :Tq] = o_q[g].T.reshape(KT, 128, Tq).astype(np.float16)
            ql = int(q_lengths[g])
            # qb (exp bias: 0 valid / -60000 masked) rides in the last slab
            oqT[KT - 1, :, Tq : Tq + JT] = np.where(
                jidx < ql, np.float16(0.0), NEG16
            )
            ocT = np.zeros((KT, 128, CTW), np.float16)
            ocT[:, :, :Tc] = o_c[g].T.reshape(KT, 128, Tc).astype(np.float16)
            ocT[0, :, Tc] = 1.0  # ones column for the denominator matmul
            in_maps[c][f"oqT{s}"] = oqT
            in_maps[c][f"ocT{s}"] = ocT
            in_maps[c][f"oqN{s}"] = np.ascontiguousarray(o_q[g].astype(np.float16))
    return in_maps


def kernel(**inputs) -> np.ndarray:
    o_c = np.asarray(inputs["o_c"], dtype=np.float32)
    o_q = np.asarray(inputs["o_q"], dtype=np.float32)
    W = np.asarray(inputs["W"], dtype=np.float32)
    q_lengths = np.asarray(inputs["q_lengths"]).astype(np.int64)
    c_lengths = np.asarray(inputs["c_lengths"]).astype(np.int64)

    from concourse.bass_utils import run_bass_kernel_spmd

    jt = [min(max(-(-int(q) // 128), 1), JT) for q in q_lengths]
    it = [min(max(-(-int(cl) // 128), 1), IT) for cl in c_lengths]
    slots = _assign_slots(jt, it)
    slot_shapes = [
        (max(jt[g] for g in grp), max(it[g] for g in grp)) for grp in slots
    ]

    in_maps = _host_inputs(o_c, o_q, W, q_lengths, slots)
    nc = _build_program(slot_shapes)

    trace = bool(int(os.environ.get("KERNEL_TRACE", "0")))
    res = run_bass_kernel_spmd(
        nc, in_maps, core_ids=list(range(N_CORES)), trace=trace
    )
    if trace:
        kernel.last_results = res

    out = np.zeros((B, Tc, H), dtype=np.float32)
    for s, grp in enumerate(slots):
        for c, g in enumerate(grp):
            dev = res.results[c][f"out{s}"]
            dvec = res.results[c][f"dvec{s}"].reshape(-1)
            cl = int(c_lengths[g])
            out[g, :cl] = dev[:cl].astype(np.float32) / dvec[:cl, None]
    return out


# revision 23
# speedup vs baseline: 1.0192x; 1.0192x over previous
"""C2Q attention kernel for 8 TRN2 NeuronCores.

Math (per batch):
    u      = (o_q @ W.T) / sqrt(H)               [Tq, H]
    score  = o_c @ u.T                           [Tc, Tq]
    prob   = softmax_j(score masked at j>=q_len) [Tc, Tq]
    out    = (prob * (i < c_len)) @ o_q          [Tc, H]

The Linear bias is dropped: it adds sum_h o_c[i,h]*b[h] to score[i, j] for
every j, i.e. a per-i constant, which softmax over j cancels exactly; the
context matmul uses raw o_q, so b never reaches the output.

Device layout choices (everything lands K-on-partitions with zero on-chip
transposes of activations):
    u computed as [o, j]  (lhsT = W.T[h, o] tile, rhs = o_qT[h, j])
    score computed TRANSPOSED e=[j, i] (lhsT = u[o, j-block], rhs = o_cT[o, i])
    exp via ACT with per-partition bias qb[j] in {0, -6e4}: masked -> exactly 0
    denominator d[1, i] = ones[j,1].T @ e  (matmul partition-reduce)
    1/d transposed to columns via K=1 matmuls, folded into context eviction
    context [i, h] = e[j, i-block].T @ o_q[j, h]   (natural output layout)
c_len row masking is applied host-side (those rows are zeroed, never read).

Ragged specialization: the program is built AFTER the inputs are known, so
per-batch tile counts jt=ceil(q_len/128), it=ceil(c_len/128) are baked in.
Batches are assigned to (core, slot) pairs; SPMD requires every core to run
the same program, so slot s uses the max (J_s, I_s) over the 8 batches
assigned to it; the assignment minimizes total padded work. Masked-out j
tiles contribute e == 0 exactly (exp bias), so skipping them is lossless;
rows i in [c_len, I*128) are computed but never read by the host gather.

IMPORTANT (measured, not theorized): DRAM parameter sizes and DMA transfer
shapes must be IDENTICAL across slots. Builds with per-slot ragged slab
widths run the whole NeuronCore ~20% slower (2.0 vs 2.4 GHz effective on
every engine, from t=0). So all slabs/DMAs are full-size; only the compute
(matmul trip counts / AP sizes) is specialized.

Slot 0 is special-cased for the DMA ramp: its Linear runs k-major with all
8 PSUM banks as accumulators (the PE has 8 runnable matmuls the moment each
(wt_k, oqT_k) pair lands), and its score runs o-outer with J concurrent
PSUM groups (each o step needs only ocT_k[o], so the PE chases the ocT DMA
tail instead of stalling on the full slab set; a PE stall here also drops
the PE to a low pstate which costs ~2us extra to ramp back).
"""

import os
import sys

import numpy as np

if "/opt/trn_rl_repo" not in sys.path:
    sys.path.insert(0, "/opt/trn_rl_repo")

B, Tc, Tq, H = 32, 512, 512, 1024
N_CORES = 8
N_SLOTS = B // N_CORES  # 4 batches per core, one per slot
KT = H // 128  # contraction tiles over h (8)
OT = H // 128  # linear-output tiles over o (8)
JT = Tq // 128
IT = Tc // 128
HB = H // 512  # free-dim blocks for context matmul (2)
SCALE = 1.0 / 32.0  # 1/sqrt(H)
QTW = Tq + 8  # uniform slab widths (see docstring)
CTW = Tc + 8


def _slot_cost(j, i):
    # PE rows: Linear 64*j*128, score j*8*i*128, d i*128, ctx i*2*j*512
    return 8192 * j + 2048 * j * i + 128 * i


def _assign_slots(jt, it):
    """Partition the B batches into N_SLOTS groups of N_CORES, minimizing
    sum over groups of _slot_cost(maxJ, maxI). Returns list of groups
    (each a list of batch indices), sorted largest-cost first."""
    idx = list(range(B))
    best = None
    for key in (
        lambda b: (jt[b], it[b]),
        lambda b: (it[b], jt[b]),
        lambda b: _slot_cost(jt[b], it[b]),
    ):
        order = sorted(idx, key=key, reverse=True)
        slots = [order[s * N_CORES : (s + 1) * N_CORES] for s in range(N_SLOTS)]

        def tot(slots):
            return sum(
                _slot_cost(max(jt[b] for b in g), max(it[b] for b in g))
                for g in slots
            )

        improved = True
        while improved:
            improved = False
            for s1 in range(N_SLOTS):
                for s2 in range(s1 + 1, N_SLOTS):
                    for a in range(N_CORES):
                        for b2 in range(N_CORES):
                            cur = tot(slots)
                            slots[s1][a], slots[s2][b2] = (
                                slots[s2][b2],
                                slots[s1][a],
                            )
                            if tot(slots) < cur:
                                improved = True
                            else:
                                slots[s1][a], slots[s2][b2] = (
                                    slots[s2][b2],
                                    slots[s1][a],
                                )
            if best is None or tot(slots) < best[0]:
                best = (tot(slots), [list(g) for g in slots])
    slots = best[1]
    slots.sort(
        key=lambda g: _slot_cost(max(jt[b] for b in g), max(it[b] for b in g)),
        reverse=True,
    )
    return slots


def _build_program(slot_shapes):
    """slot_shapes: list of (J, I) per slot; one batch per core per slot."""
    import concourse.bacc as bacc
    import concourse.mybir as mybir
    import concourse.tile as tile

    f32 = mybir.dt.float32
    f16 = mybir.dt.float16
    Copy = mybir.ActivationFunctionType.Copy

    nc = bacc.Bacc("TRN2", debug=False)

    oqT_d, ocT_d, oqN_d, out_d, d_d = [], [], [], [], []
    for s in range(len(slot_shapes)):
        oqT_d.append(
            nc.declare_dram_parameter(f"oqT{s}", [KT, 128, QTW], f16, isOutput=False)
        )
        ocT_d.append(
            nc.declare_dram_parameter(f"ocT{s}", [KT, 128, CTW], f16, isOutput=False)
        )
        oqN_d.append(
            nc.declare_dram_parameter(f"oqN{s}", [Tq, H], f16, isOutput=False)
        )
        out_d.append(
            nc.declare_dram_parameter(f"out{s}", [Tc, H], f16, isOutput=True)
        )
        d_d.append(
            nc.declare_dram_parameter(f"dvec{s}", [1, Tc], f32, isOutput=True)
        )
    wt_d = nc.declare_dram_parameter("wt", [KT, 128, H], f16, isOutput=False)

    with tile.TileContext(nc) as tc:
        with (
            tc.tile_pool(name="const", bufs=1) as cpool,
            tc.tile_pool(name="inp", bufs=2) as ipool,
            tc.tile_pool(name="work", bufs=1) as wpool,
            tc.tile_pool(name="outp", bufs=3) as opool,
            tc.tile_pool(name="ps_u", bufs=2, space="PSUM") as ps_u,
            tc.tile_pool(name="ps_s", bufs=2, space="PSUM") as ps_s,
            tc.tile_pool(name="ps_c", bufs=3, space="PSUM") as ps_c,
            tc.tile_pool(name="ps_d", bufs=1, space="PSUM") as ps_d,
        ):
            ones_s = cpool.tile([1, 1], f32)
            nc.vector.memset(ones_s, 1.0)

            # PE warm-up: a short dependency-free matmul chain right after
            # boot ramps the PE out of its low pstate before the first real
            # operands land (~11us in); results are discarded. The dummy
            # PSUM group (dmisc bank) is free until slot 0's k-major Linear
            # reaches its 8th accumulator (~13us).
            dummy = cpool.tile([128, 512], f16, tag="dummy", name="dummy")
            nc.vector.memset(dummy, 0.0)
            dps_w = ps_d.tile([128, 512], f32, tag="dmisc", name="warm")
            for w in range(10):
                nc.tensor.matmul(
                    dps_w,
                    dummy[:, :128],
                    dummy[:, :512],
                    start=True,
                    stop=True,
                    skip_group_check=True,
                )

            wt_k = [cpool.tile([128, H], f16, tag=f"wt{k}", name=f"wt{k}") for k in range(KT)]

            def d_accum(e_tiles, dps, ic, s):
                # pre-add e pairs on DVE (16-bit 2x rate) so the PE's
                # partition-reduce runs over half as many rows
                J = len(e_tiles)
                groups = []
                k = 0
                while k + 1 < J:
                    ep = wpool.tile([128, ic], mybir.dt.float16, tag=f"ep{k//2}", name=f"ep{k//2}_{s}")
                    nc.vector.tensor_tensor(
                        out=ep, in0=e_tiles[k], in1=e_tiles[k + 1],
                        op=mybir.AluOpType.add,
                    )
                    groups.append(ep)
                    k += 2
                if k < J:
                    groups.append(e_tiles[k])
                ones_c = groups  # accumulate ones.T @ each group
                for gi, g in enumerate(groups):
                    nc.tensor.matmul(
                        dps,
                        ones,
                        g,
                        start=(gi == 0),
                        stop=(gi == len(groups) - 1),
                        skip_group_check=True,
                    )

            def u_evict(o, dst, src):
                # alternate engines so the evict chain is ~2x faster; the
                # score matmuls consume u[o] in order
                if o % 2 == 0:
                    nc.vector.tensor_copy(out=dst, in_=src)
                else:
                    nc.scalar.activation(out=dst, in_=src, func=Copy)

            for s, (J, I) in enumerate(slot_shapes):
                qc, ic = J * 128, I * 128
                oqT_k = [ipool.tile([128, QTW], f16, tag=f"oqT{k}", name=f"oqT{k}_{s}") for k in range(KT)]
                ocT_k = [ipool.tile([128, CTW], f16, tag=f"ocT{k}", name=f"ocT{k}_{s}") for k in range(KT)]
                oqN = ipool.tile([128, JT, H], f16, tag="oqN", name=f"oqN_{s}")
                qb = oqT_k[KT - 1][:, Tq : Tq + J]
                ones = ocT_k[0][:, Tc : Tc + 1]
                for k in range(KT):
                    if s == 0:
                        nc.sync.dma_start(out=wt_k[k], in_=wt_d[k])
                    nc.sync.dma_start(out=oqT_k[k], in_=oqT_d[s][k])
                for k in range(KT):
                    nc.sync.dma_start(out=ocT_k[k], in_=ocT_d[s][k])
                for j in range(JT):
                    nc.sync.dma_start(
                        out=oqN[:, j, :], in_=oqN_d[s][j * 128 : (j + 1) * 128, :]
                    )

                # ---- Linear: u[o, j] = W@o_q.T  (the 1/32 scale rides in
                # the Exp activation's scale argument) ----
                u = wpool.tile([128, OT, qc], f16, tag="u", name=f"u_{s}")
                e_tiles = []
                dps = ps_d.tile([1, Tc], f32, tag="dmisc", name=f"dps_{s}")
                if s == 0:
                    ups_o = [
                        ps_u.tile([128, qc], f32, tag="ups", name="ups_a"),
                        ps_u.tile([128, qc], f32, tag="ups", name="ups_b"),
                        ps_s.tile([128, qc], f32, tag="sps", name="ups_c"),
                        ps_s.tile([128, qc], f32, tag="sps", name="ups_d"),
                        ps_c.tile([128, qc], f32, tag="cps", name="ups_e"),
                        ps_c.tile([128, qc], f32, tag="cps", name="ups_f"),
                        ps_c.tile([128, qc], f32, tag="cps", name="ups_g"),
                        ps_d.tile([128, qc], f32, tag="dmisc", name="ups_h"),
                    ]
                    for k in range(KT):
                        for o in range(OT):
                            nc.tensor.matmul(
                                ups_o[o],
                                wt_k[k][:, o * 128 : (o + 1) * 128],
                                oqT_k[k][:, :qc],
                                start=(k == 0),
                                stop=(k == KT - 1),
                            )
                    for o in range(OT):
                        u_evict(o, u[:, o, :], ups_o[o])

                    # ---- score o-outer: each o step needs only ocT_k[o],
                    # so the PE chases the ocT DMA tail ----
                    sps_j = [
                        ps_s.tile([128, ic], f32, tag="sps", name="sps_j0"),
                        ps_s.tile([128, ic], f32, tag="sps", name="sps_j1"),
                        ps_u.tile([128, ic], f32, tag="ups", name="sps_j2"),
                        ps_u.tile([128, ic], f32, tag="ups", name="sps_j3"),
                    ][:J]
                    for o in range(OT):
                        for jt_ in range(J):
                            nc.tensor.matmul(
                                sps_j[jt_],
                                u[:, o, jt_ * 128 : (jt_ + 1) * 128],
                                ocT_k[o][:, :ic],
                                start=(o == 0),
                                stop=(o == OT - 1),
                            )
                    for jt_ in range(J):
                        e = wpool.tile([128, ic], f16, tag=f"e{jt_}", name=f"e{jt_}_{s}")
                        nc.scalar.activation(
                            out=e,
                            in_=sps_j[jt_],
                            func=mybir.ActivationFunctionType.Exp,
                            bias=qb[:, jt_ : jt_ + 1],
                            scale=SCALE,
                        )
                        e_tiles.append(e)
                    d_accum(e_tiles, dps, ic, s)
                else:
                    for o in range(OT):
                        ups = ps_u.tile([128, qc], f32, tag="ups")
                        for k in range(KT):
                            nc.tensor.matmul(
                                ups,
                                wt_k[k][:, o * 128 : (o + 1) * 128],
                                oqT_k[k][:, :qc],
                                start=(k == 0),
                                stop=(k == KT - 1),
                            )
                        u_evict(o, u[:, o, :], ups)

                    # ---- score jt-outer + exp, denominator accumulation
                    # interleaved one step behind so its chain latency hides
                    for jt_ in range(J):
                        sps = ps_s.tile([128, ic], f32, tag="sps")
                        for o in range(OT):
                            nc.tensor.matmul(
                                sps,
                                u[:, o, jt_ * 128 : (jt_ + 1) * 128],
                                ocT_k[o][:, :ic],
                                start=(o == 0),
                                stop=(o == OT - 1),
                            )
                        e = wpool.tile([128, ic], f16, tag=f"e{jt_}", name=f"e{jt_}_{s}")
                        nc.scalar.activation(
                            out=e,
                            in_=sps,
                            func=mybir.ActivationFunctionType.Exp,
                            bias=qb[:, jt_ : jt_ + 1],
                            scale=SCALE,
                        )
                        e_tiles.append(e)
                    d_accum(e_tiles, dps, ic, s)

                osb_tiles = {}

                def ctx_group(itb, hb, J=J, s=s, e_tiles=e_tiles, oqN=oqN, osb_tiles=osb_tiles):
                    if itb not in osb_tiles:
                        osb_tiles[itb] = opool.tile(
                            [128, H], f16, tag="osb", name=f"osb{itb}_{s}"
                        )
                    cps = ps_c.tile([128, 512], f32, tag="cps", name=f"cps{itb}{hb}_{s}")
                    for jt_ in range(J):
                        nc.tensor.matmul(
                            cps,
                            e_tiles[jt_][:, itb * 128 : (itb + 1) * 128],
                            oqN[:, jt_, hb * 512 : (hb + 1) * 512],
                            start=(jt_ == 0),
                            stop=(jt_ == J - 1),
                        )
                    return cps

                def ctx_evict(itb, hb, cps, s=s, osb_tiles=osb_tiles, out=out_d[s]):
                    # unnormalized context; the host divides by d (shipped
                    # separately), so the evict is a plain copy and can
                    # alternate engines like u_evict does
                    osb = osb_tiles[itb]
                    if hb % 2 == 0:
                        nc.vector.tensor_copy(
                            out=osb[:, hb * 512 : (hb + 1) * 512], in_=cps
                        )
                    else:
                        nc.scalar.activation(
                            out=osb[:, hb * 512 : (hb + 1) * 512], in_=cps, func=Copy
                        )
                    nc.sync.dma_start(
                        out=out[
                            itb * 128 : (itb + 1) * 128, hb * 512 : (hb + 1) * 512
                        ],
                        in_=osb[:, hb * 512 : (hb + 1) * 512],
                    )

                # d ships to the host (the division happens there); the
                # copy+DMA drain while the first ctx groups run
                cps00 = ctx_group(0, 0)
                dsb = wpool.tile([1, Tc], f32, tag="dsb", name=f"dsb_{s}")
                nc.vector.tensor_copy(out=dsb, in_=dps)
                nc.sync.dma_start(out=d_d[s][:, :], in_=dsb)

                cps01 = ctx_group(0, 1)
                ctx_evict(0, 0, cps00)
                ctx_evict(0, 1, cps01)
                for itb in range(1, I):
                    for hb in range(HB):
                        cps = ctx_group(itb, hb)
                        ctx_evict(itb, hb, cps)

    nc.compile()
    return nc


def _host_inputs(o_c, o_q, W, q_lengths, slots):
    """Per-core input maps (host-side sharding + re-layout). All slabs are
    full-size (uniform DMA shapes across slots; see module docstring)."""
    NEG16 = np.float16(-60000.0)  # exp(x - 60000) == 0 exactly in fp32
    wt_host = np.ascontiguousarray(W.T.reshape(KT, 128, H).astype(np.float16))
    jidx = np.arange(JT)[None, :] * 128 + np.arange(128)[:, None]  # [128, JT]
    in_maps = [{"wt": wt_host} for _ in range(N_CORES)]
    for s, grp in enumerate(slots):
        for c, g in enumerate(grp):
            oqT = np.zeros((KT, 128, QTW), np.float16)
            oqT[:, :, :Tq] = o_q[g].T.reshape(KT, 128, Tq).astype(np.float16)
            ql = int(q_lengths[g])
            # qb (exp bias: 0 valid / -60000 masked) rides in the last slab
            oqT[KT - 1, :, Tq : Tq + JT] = np.where(
                jidx < ql, np.float16(0.0), NEG16
            )
            ocT = np.zeros((KT, 128, CTW), np.float16)
            ocT[:, :, :Tc] = o_c[g].T.reshape(KT, 128, Tc).astype(np.float16)
            ocT[0, :, Tc] = 1.0  # ones column for the denominator matmul
            in_maps[c][f"oqT{s}"] = oqT
            in_maps[c][f"ocT{s}"] = ocT
            in_maps[c][f"oqN{s}"] = np.ascontiguousarray(o_q[g].astype(np.float16))
    return in_maps


def kernel(**inputs) -> np.ndarray:
    o_c = np.asarray(inputs["o_c"], dtype=np.float32)
    o_q = np.asarray(inputs["o_q"], dtype=np.float32)
    W = np.asarray(inputs["W"], dtype=np.float32)
    q_lengths = np.asarray(inputs["q_lengths"]).astype(np.int64)
    c_lengths = np.asarray(inputs["c_lengths"]).astype(np.int64)

    from concourse.bass_utils import run_bass_kernel_spmd

    jt = [min(max(-(-int(q) // 128), 1), JT) for q in q_lengths]
    it = [min(max(-(-int(cl) // 128), 1), IT) for cl in c_lengths]
    slots = _assign_slots(jt, it)
    slot_shapes = [
        (max(jt[g] for g in grp), max(it[g] for g in grp)) for grp in slots
    ]

    in_maps = _host_inputs(o_c, o_q, W, q_lengths, slots)
    nc = _build_program(slot_shapes)

    trace = bool(int(os.environ.get("KERNEL_TRACE", "0")))
    res = run_bass_kernel_spmd(
        nc, in_maps, core_ids=list(range(N_CORES)), trace=trace
    )
    if trace:
        kernel.last_results = res

    out = np.zeros((B, Tc, H), dtype=np.float32)
    for s, grp in enumerate(slots):
        for c, g in enumerate(grp):
            dev = res.results[c][f"out{s}"]
            dvec = res.results[c][f"dvec{s}"].reshape(-1)
            cl = int(c_lengths[g])
            out[g, :cl] = dev[:cl].astype(np.float32) / dvec[:cl, None]
    return out


# revision 24
# speedup vs baseline: 1.0311x; 1.0117x over previous
"""C2Q attention kernel for 8 TRN2 NeuronCores.

Math (per batch):
    u      = (o_q @ W.T) / sqrt(H)               [Tq, H]
    score  = o_c @ u.T                           [Tc, Tq]
    prob   = softmax_j(score masked at j>=q_len) [Tc, Tq]
    out    = (prob * (i < c_len)) @ o_q          [Tc, H]

The Linear bias is dropped: it adds sum_h o_c[i,h]*b[h] to score[i, j] for
every j, i.e. a per-i constant, which softmax over j cancels exactly; the
context matmul uses raw o_q, so b never reaches the output.

Device layout choices (everything lands K-on-partitions with zero on-chip
transposes of activations):
    u computed as [o, j]  (lhsT = W.T[h, o] tile, rhs = o_qT[h, j])
    score computed TRANSPOSED e=[j, i] (lhsT = u[o, j-block], rhs = o_cT[o, i])
    exp via ACT with per-partition bias qb[j] in {0, -6e4}: masked -> exactly 0
    denominator d[1, i] = ones[j,1].T @ e  (matmul partition-reduce)
    1/d transposed to columns via K=1 matmuls, folded into context eviction
    context [i, h] = e[j, i-block].T @ o_q[j, h]   (natural output layout)
c_len row masking is applied host-side (those rows are zeroed, never read).

Ragged specialization: the program is built AFTER the inputs are known, so
per-batch tile counts jt=ceil(q_len/128), it=ceil(c_len/128) are baked in.
Batches are assigned to (core, slot) pairs; SPMD requires every core to run
the same program, so slot s uses the max (J_s, I_s) over the 8 batches
assigned to it; the assignment minimizes total padded work. Masked-out j
tiles contribute e == 0 exactly (exp bias), so skipping them is lossless;
rows i in [c_len, I*128) are computed but never read by the host gather.

IMPORTANT (measured, not theorized): DRAM parameter sizes and DMA transfer
shapes must be IDENTICAL across slots. Builds with per-slot ragged slab
widths run the whole NeuronCore ~20% slower (2.0 vs 2.4 GHz effective on
every engine, from t=0). So all slabs/DMAs are full-size; only the compute
(matmul trip counts / AP sizes) is specialized.

Slot 0 is special-cased for the DMA ramp: its Linear runs k-major with all
8 PSUM banks as accumulators (the PE has 8 runnable matmuls the moment each
(wt_k, oqT_k) pair lands), and its score runs o-outer with J concurrent
PSUM groups (each o step needs only ocT_k[o], so the PE chases the ocT DMA
tail instead of stalling on the full slab set; a PE stall here also drops
the PE to a low pstate which costs ~2us extra to ramp back).
"""

import os
import sys

import numpy as np

if "/opt/trn_rl_repo" not in sys.path:
    sys.path.insert(0, "/opt/trn_rl_repo")

B, Tc, Tq, H = 32, 512, 512, 1024
N_CORES = 8
N_SLOTS = B // N_CORES  # 4 batches per core, one per slot
KT = H // 128  # contraction tiles over h (8)
OT = H // 128  # linear-output tiles over o (8)
JT = Tq // 128
IT = Tc // 128
HB = H // 512  # free-dim blocks for context matmul (2)
SCALE = 1.0 / 32.0  # 1/sqrt(H)
QTW = Tq + 8  # uniform slab widths (see docstring)
CTW = Tc + 8


def _slot_cost(j, i):
    # PE rows: Linear 64*j*128, score j*8*i*128, d i*128, ctx i*2*j*512
    return 8192 * j + 2048 * j * i + 128 * i


def _assign_slots(jt, it):
    """Partition the B batches into N_SLOTS groups of N_CORES, minimizing
    sum over groups of _slot_cost(maxJ, maxI). Returns list of groups
    (each a list of batch indices), sorted largest-cost first."""
    idx = list(range(B))
    best = None
    for key in (
        lambda b: (jt[b], it[b]),
        lambda b: (it[b], jt[b]),
        lambda b: _slot_cost(jt[b], it[b]),
    ):
        order = sorted(idx, key=key, reverse=True)
        slots = [order[s * N_CORES : (s + 1) * N_CORES] for s in range(N_SLOTS)]

        def tot(slots):
            return sum(
                _slot_cost(max(jt[b] for b in g), max(it[b] for b in g))
                for g in slots
            )

        improved = True
        while improved:
            improved = False
            for s1 in range(N_SLOTS):
                for s2 in range(s1 + 1, N_SLOTS):
                    for a in range(N_CORES):
                        for b2 in range(N_CORES):
                            cur = tot(slots)
                            slots[s1][a], slots[s2][b2] = (
                                slots[s2][b2],
                                slots[s1][a],
                            )
                            if tot(slots) < cur:
                                improved = True
                            else:
                                slots[s1][a], slots[s2][b2] = (
                                    slots[s2][b2],
                                    slots[s1][a],
                                )
            if best is None or tot(slots) < best[0]:
                best = (tot(slots), [list(g) for g in slots])
    slots = best[1]
    slots.sort(
        key=lambda g: _slot_cost(max(jt[b] for b in g), max(it[b] for b in g)),
        reverse=True,
    )
    return slots


def _build_program(slot_shapes):
    """slot_shapes: list of (J, I) per slot; one batch per core per slot."""
    import concourse.bacc as bacc
    import concourse.mybir as mybir
    import concourse.tile as tile

    f32 = mybir.dt.float32
    f16 = mybir.dt.float16
    Copy = mybir.ActivationFunctionType.Copy

    nc = bacc.Bacc("TRN2", debug=False)

    oqT_d, ocT_d, oqN_d, out_d, d_d = [], [], [], [], []
    for s in range(len(slot_shapes)):
        oqT_d.append(
            nc.declare_dram_parameter(f"oqT{s}", [KT, 128, QTW], f16, isOutput=False)
        )
        ocT_d.append(
            nc.declare_dram_parameter(f"ocT{s}", [KT, 128, CTW], f16, isOutput=False)
        )
        oqN_d.append(
            nc.declare_dram_parameter(f"oqN{s}", [Tq, H], f16, isOutput=False)
        )
        out_d.append(
            nc.declare_dram_parameter(f"out{s}", [Tc, H], f16, isOutput=True)
        )
        d_d.append(
            nc.declare_dram_parameter(f"dvec{s}", [1, Tc], f32, isOutput=True)
        )
    wt_d = nc.declare_dram_parameter("wt", [KT, 128, H], f16, isOutput=False)

    with tile.TileContext(nc) as tc:
        with (
            tc.tile_pool(name="const", bufs=1) as cpool,
            tc.tile_pool(name="inp", bufs=3) as ipool,
            tc.tile_pool(name="work", bufs=1) as wpool,
            tc.tile_pool(name="outp", bufs=4) as opool,
            tc.tile_pool(name="ps_u", bufs=2, space="PSUM") as ps_u,
            tc.tile_pool(name="ps_s", bufs=2, space="PSUM") as ps_s,
            tc.tile_pool(name="ps_c", bufs=3, space="PSUM") as ps_c,
            tc.tile_pool(name="ps_d", bufs=1, space="PSUM") as ps_d,
        ):
            ones_s = cpool.tile([1, 1], f32)
            nc.vector.memset(ones_s, 1.0)

            # PE warm-up: a short dependency-free matmul chain right after
            # boot ramps the PE out of its low pstate before the first real
            # operands land (~11us in); results are discarded. The dummy
            # PSUM group (dmisc bank) is free until slot 0's k-major Linear
            # reaches its 8th accumulator (~13us).
            dummy = cpool.tile([128, 512], f16, tag="dummy", name="dummy")
            nc.vector.memset(dummy, 0.0)
            dps_w = ps_d.tile([128, 512], f32, tag="dmisc", name="warm")
            for w in range(10):
                nc.tensor.matmul(
                    dps_w,
                    dummy[:, :128],
                    dummy[:, :512],
                    start=True,
                    stop=True,
                    skip_group_check=True,
                )

            wt_k = [cpool.tile([128, H], f16, tag=f"wt{k}", name=f"wt{k}") for k in range(KT)]

            def d_accum(e_tiles, dps, ic, s):
                # pre-add e pairs on DVE (16-bit 2x rate) so the PE's
                # partition-reduce runs over half as many rows
                J = len(e_tiles)
                groups = []
                k = 0
                while k + 1 < J:
                    ep = wpool.tile([128, ic], mybir.dt.float16, tag=f"ep{k//2}", name=f"ep{k//2}_{s}")
                    nc.vector.tensor_tensor(
                        out=ep, in0=e_tiles[k], in1=e_tiles[k + 1],
                        op=mybir.AluOpType.add,
                    )
                    groups.append(ep)
                    k += 2
                if k < J:
                    groups.append(e_tiles[k])
                for gi, g in enumerate(groups):
                    nc.tensor.matmul(
                        dps,
                        ones,
                        g,
                        start=(gi == 0),
                        stop=(gi == len(groups) - 1),
                        skip_group_check=True,
                    )

            def u_evict(o, dst, src):
                # alternate engines so the evict chain is ~2x faster; the
                # score matmuls consume u[o] in order
                if o % 2 == 0:
                    nc.vector.tensor_copy(out=dst, in_=src)
                else:
                    nc.scalar.activation(out=dst, in_=src, func=Copy)

            for s, (J, I) in enumerate(slot_shapes):
                qc, ic = J * 128, I * 128
                oqT_k = [ipool.tile([128, QTW], f16, tag=f"oqT{k}", name=f"oqT{k}_{s}") for k in range(KT)]
                ocT_k = [ipool.tile([128, CTW], f16, tag=f"ocT{k}", name=f"ocT{k}_{s}") for k in range(KT)]
                oqN = ipool.tile([128, JT, H], f16, tag="oqN", name=f"oqN_{s}")
                qb = oqT_k[KT - 1][:, Tq : Tq + J]
                ones = ocT_k[0][:, Tc : Tc + 1]
                for k in range(KT):
                    if s == 0:
                        nc.sync.dma_start(out=wt_k[k], in_=wt_d[k])
                    nc.sync.dma_start(out=oqT_k[k], in_=oqT_d[s][k])
                for k in range(KT):
                    nc.sync.dma_start(out=ocT_k[k], in_=ocT_d[s][k])
                for j in range(JT):
                    nc.sync.dma_start(
                        out=oqN[:, j, :], in_=oqN_d[s][j * 128 : (j + 1) * 128, :]
                    )

                # ---- Linear: u[o, j] = W@o_q.T  (the 1/32 scale rides in
                # the Exp activation's scale argument) ----
                u = wpool.tile([128, OT, qc], f16, tag="u", name=f"u_{s}")
                e_tiles = []
                dps = ps_d.tile([1, Tc], f32, tag="dmisc", name=f"dps_{s}")
                if s == 0:
                    ups_o = [
                        ps_u.tile([128, qc], f32, tag="ups", name="ups_a"),
                        ps_u.tile([128, qc], f32, tag="ups", name="ups_b"),
                        ps_s.tile([128, qc], f32, tag="sps", name="ups_c"),
                        ps_s.tile([128, qc], f32, tag="sps", name="ups_d"),
                        ps_c.tile([128, qc], f32, tag="cps", name="ups_e"),
                        ps_c.tile([128, qc], f32, tag="cps", name="ups_f"),
                        ps_c.tile([128, qc], f32, tag="cps", name="ups_g"),
                        ps_d.tile([128, qc], f32, tag="dmisc", name="ups_h"),
                    ]
                    for k in range(KT):
                        for o in range(OT):
                            nc.tensor.matmul(
                                ups_o[o],
                                wt_k[k][:, o * 128 : (o + 1) * 128],
                                oqT_k[k][:, :qc],
                                start=(k == 0),
                                stop=(k == KT - 1),
                            )
                    for o in range(OT):
                        u_evict(o, u[:, o, :], ups_o[o])

                    # ---- score o-outer: each o step needs only ocT_k[o],
                    # so the PE chases the ocT DMA tail ----
                    sps_j = [
                        ps_s.tile([128, ic], f32, tag="sps", name="sps_j0"),
                        ps_s.tile([128, ic], f32, tag="sps", name="sps_j1"),
                        ps_u.tile([128, ic], f32, tag="ups", name="sps_j2"),
                        ps_u.tile([128, ic], f32, tag="ups", name="sps_j3"),
                    ][:J]
                    for o in range(OT):
                        for jt_ in range(J):
                            nc.tensor.matmul(
                                sps_j[jt_],
                                u[:, o, jt_ * 128 : (jt_ + 1) * 128],
                                ocT_k[o][:, :ic],
                                start=(o == 0),
                                stop=(o == OT - 1),
                            )
                    for jt_ in range(J):
                        e = wpool.tile([128, ic], f16, tag=f"e{jt_}", name=f"e{jt_}_{s}")
                        nc.scalar.activation(
                            out=e,
                            in_=sps_j[jt_],
                            func=mybir.ActivationFunctionType.Exp,
                            bias=qb[:, jt_ : jt_ + 1],
                            scale=SCALE,
                        )
                        e_tiles.append(e)
                    d_accum(e_tiles, dps, ic, s)
                else:
                    for o in range(OT):
                        ups = ps_u.tile([128, qc], f32, tag="ups")
                        for k in range(KT):
                            nc.tensor.matmul(
                                ups,
                                wt_k[k][:, o * 128 : (o + 1) * 128],
                                oqT_k[k][:, :qc],
                                start=(k == 0),
                                stop=(k == KT - 1),
                            )
                        u_evict(o, u[:, o, :], ups)

                    # ---- score jt-outer + exp, denominator accumulation
                    # interleaved one step behind so its chain latency hides
                    for jt_ in range(J):
                        sps = ps_s.tile([128, ic], f32, tag="sps")
                        for o in range(OT):
                            nc.tensor.matmul(
                                sps,
                                u[:, o, jt_ * 128 : (jt_ + 1) * 128],
                                ocT_k[o][:, :ic],
                                start=(o == 0),
                                stop=(o == OT - 1),
                            )
                        e = wpool.tile([128, ic], f16, tag=f"e{jt_}", name=f"e{jt_}_{s}")
                        nc.scalar.activation(
                            out=e,
                            in_=sps,
                            func=mybir.ActivationFunctionType.Exp,
                            bias=qb[:, jt_ : jt_ + 1],
                            scale=SCALE,
                        )
                        e_tiles.append(e)
                    d_accum(e_tiles, dps, ic, s)

                osb_tiles = {}

                def ctx_group(itb, hb, J=J, s=s, e_tiles=e_tiles, oqN=oqN, osb_tiles=osb_tiles):
                    if itb not in osb_tiles:
                        osb_tiles[itb] = opool.tile(
                            [128, H], f16, tag="osb", name=f"osb{itb}_{s}"
                        )
                    cps = ps_c.tile([128, 512], f32, tag="cps", name=f"cps{itb}{hb}_{s}")
                    for jt_ in range(J):
                        nc.tensor.matmul(
                            cps,
                            e_tiles[jt_][:, itb * 128 : (itb + 1) * 128],
                            oqN[:, jt_, hb * 512 : (hb + 1) * 512],
                            start=(jt_ == 0),
                            stop=(jt_ == J - 1),
                        )
                    return cps

                def ctx_evict(itb, hb, cps, s=s, osb_tiles=osb_tiles, out=out_d[s]):
                    # unnormalized context; the host divides by d (shipped
                    # separately), so the evict is a plain copy and can
                    # alternate engines like u_evict does
                    osb = osb_tiles[itb]
                    if hb % 2 == 0:
                        nc.vector.tensor_copy(
                            out=osb[:, hb * 512 : (hb + 1) * 512], in_=cps
                        )
                    else:
                        nc.scalar.activation(
                            out=osb[:, hb * 512 : (hb + 1) * 512], in_=cps, func=Copy
                        )
                    nc.sync.dma_start(
                        out=out[
                            itb * 128 : (itb + 1) * 128, hb * 512 : (hb + 1) * 512
                        ],
                        in_=osb[:, hb * 512 : (hb + 1) * 512],
                    )

                # d ships to the host (the division happens there); the
                # copy+DMA drain while the first ctx groups run
                cps00 = ctx_group(0, 0)
                dsb = wpool.tile([1, Tc], f32, tag="dsb", name=f"dsb_{s}")
                nc.vector.tensor_copy(out=dsb, in_=dps)
                nc.sync.dma_start(out=d_d[s][:, :], in_=dsb)

                cps01 = ctx_group(0, 1)
                ctx_evict(0, 0, cps00)
                ctx_evict(0, 1, cps01)
                for itb in range(1, I):
                    for hb in range(HB):
                        cps = ctx_group(itb, hb)
                        ctx_evict(itb, hb, cps)

    nc.compile()
    return nc


def _host_inputs(o_c, o_q, W, q_lengths, slots):
    """Per-core input maps (host-side sharding + re-layout). All slabs are
    full-size (uniform DMA shapes across slots; see module docstring)."""
    NEG16 = np.float16(-60000.0)  # exp(x - 60000) == 0 exactly in fp32
    wt_host = np.ascontiguousarray(W.T.reshape(KT, 128, H).astype(np.float16))
    jidx = np.arange(JT)[None, :] * 128 + np.arange(128)[:, None]  # [128, JT]
    in_maps = [{"wt": wt_host} for _ in range(N_CORES)]
    for s, grp in enumerate(slots):
        for c, g in enumerate(grp):
            oqT = np.zeros((KT, 128, QTW), np.float16)
            oqT[:, :, :Tq] = o_q[g].T.reshape(KT, 128, Tq).astype(np.float16)
            ql = int(q_lengths[g])
            # qb (exp bias: 0 valid / -60000 masked) rides in the last slab
            oqT[KT - 1, :, Tq : Tq + JT] = np.where(
                jidx < ql, np.float16(0.0), NEG16
            )
            ocT = np.zeros((KT, 128, CTW), np.float16)
            ocT[:, :, :Tc] = o_c[g].T.reshape(KT, 128, Tc).astype(np.float16)
            ocT[0, :, Tc] = 1.0  # ones column for the denominator matmul
            in_maps[c][f"oqT{s}"] = oqT
            in_maps[c][f"ocT{s}"] = ocT
            in_maps[c][f"oqN{s}"] = np.ascontiguousarray(o_q[g].astype(np.float16))
    return in_maps


def kernel(**inputs) -> np.ndarray:
    o_c = np.asarray(inputs["o_c"], dtype=np.float32)
    o_q = np.asarray(inputs["o_q"], dtype=np.float32)
    W = np.asarray(inputs["W"], dtype=np.float32)
    q_lengths = np.asarray(inputs["q_lengths"]).astype(np.int64)
    c_lengths = np.asarray(inputs["c_lengths"]).astype(np.int64)

    from concourse.bass_utils import run_bass_kernel_spmd

    jt = [min(max(-(-int(q) // 128), 1), JT) for q in q_lengths]
    it = [min(max(-(-int(cl) // 128), 1), IT) for cl in c_lengths]
    slots = _assign_slots(jt, it)
    slot_shapes = [
        (max(jt[g] for g in grp), max(it[g] for g in grp)) for grp in slots
    ]

    in_maps = _host_inputs(o_c, o_q, W, q_lengths, slots)
    nc = _build_program(slot_shapes)

    trace = bool(int(os.environ.get("KERNEL_TRACE", "0")))
    res = run_bass_kernel_spmd(
        nc, in_maps, core_ids=list(range(N_CORES)), trace=trace
    )
    if trace:
        kernel.last_results = res

    out = np.zeros((B, Tc, H), dtype=np.float32)
    for s, grp in enumerate(slots):
        for c, g in enumerate(grp):
            dev = res.results[c][f"out{s}"]
            dvec = res.results[c][f"dvec{s}"].reshape(-1)
            cl = int(c_lengths[g])
            out[g, :cl] = dev[:cl].astype(np.float32) / dvec[:cl, None]
    return out


# revision 25
# speedup vs baseline: 1.0506x; 1.0189x over previous
"""C2Q attention kernel for 8 TRN2 NeuronCores.

Math (per batch):
    u      = (o_q @ W.T) / sqrt(H)               [Tq, H]
    score  = o_c @ u.T                           [Tc, Tq]
    prob   = softmax_j(score masked at j>=q_len) [Tc, Tq]
    out    = (prob * (i < c_len)) @ o_q          [Tc, H]

The Linear bias is dropped: it adds sum_h o_c[i,h]*b[h] to score[i, j] for
every j, i.e. a per-i constant, which softmax over j cancels exactly; the
context matmul uses raw o_q, so b never reaches the output.

Device layout choices (everything lands K-on-partitions with zero on-chip
transposes of activations):
    u computed as [o, j]  (lhsT = W.T[h, o] tile, rhs = o_qT[h, j])
    score computed TRANSPOSED e=[j, i] (lhsT = u[o, j-block], rhs = o_cT[o, i])
    exp via ACT with per-partition bias qb[j] in {0, -6e4}: masked -> exactly 0
    denominator d[1, i] = ones[j,1].T @ e  (matmul partition-reduce)
    1/d transposed to columns via K=1 matmuls, folded into context eviction
    context [i, h] = e[j, i-block].T @ o_q[j, h]   (natural output layout)
c_len row masking is applied host-side (those rows are zeroed, never read).

Ragged specialization: the program is built AFTER the inputs are known, so
per-batch tile counts jt=ceil(q_len/128), it=ceil(c_len/128) are baked in.
Batches are assigned to (core, slot) pairs; SPMD requires every core to run
the same program, so slot s uses the max (J_s, I_s) over the 8 batches
assigned to it; the assignment minimizes total padded work. Masked-out j
tiles contribute e == 0 exactly (exp bias), so skipping them is lossless;
rows i in [c_len, I*128) are computed but never read by the host gather.

IMPORTANT (measured, not theorized): DRAM parameter sizes and DMA transfer
shapes must be IDENTICAL across slots. Builds with per-slot ragged slab
widths run the whole NeuronCore ~20% slower (2.0 vs 2.4 GHz effective on
every engine, from t=0). So all slabs/DMAs are full-size; only the compute
(matmul trip counts / AP sizes) is specialized.

Slot 0 is special-cased for the DMA ramp: its Linear runs k-major with all
8 PSUM banks as accumulators (the PE has 8 runnable matmuls the moment each
(wt_k, oqT_k) pair lands), and its score runs o-outer with J concurrent
PSUM groups (each o step needs only ocT_k[o], so the PE chases the ocT DMA
tail instead of stalling on the full slab set; a PE stall here also drops
the PE to a low pstate which costs ~2us extra to ramp back).
"""

import os
import sys

import numpy as np

if "/opt/trn_rl_repo" not in sys.path:
    sys.path.insert(0, "/opt/trn_rl_repo")

B, Tc, Tq, H = 32, 512, 512, 1024
N_CORES = 8
N_SLOTS = B // N_CORES  # 4 batches per core, one per slot
KT = H // 128  # contraction tiles over h (8)
OT = H // 128  # linear-output tiles over o (8)
JT = Tq // 128
IT = Tc // 128
HB = H // 512  # free-dim blocks for context matmul (2)
SCALE = 1.0 / 32.0  # 1/sqrt(H)
QTW = Tq + 8  # uniform slab widths (see docstring)
CTW = Tc + 8


def _slot_cost(j, i):
    # PE rows: Linear 64*j*128, score j*8*i*128, d i*128, ctx i*2*j*512
    return 8192 * j + 2048 * j * i + 128 * i


def _assign_slots(jt, it):
    """Partition the B batches into N_SLOTS groups of N_CORES, minimizing
    sum over groups of _slot_cost(maxJ, maxI). Returns list of groups
    (each a list of batch indices), sorted largest-cost first."""
    idx = list(range(B))
    best = None
    for key in (
        lambda b: (jt[b], it[b]),
        lambda b: (it[b], jt[b]),
        lambda b: _slot_cost(jt[b], it[b]),
    ):
        order = sorted(idx, key=key, reverse=True)
        slots = [order[s * N_CORES : (s + 1) * N_CORES] for s in range(N_SLOTS)]

        def tot(slots):
            return sum(
                _slot_cost(max(jt[b] for b in g), max(it[b] for b in g))
                for g in slots
            )

        improved = True
        while improved:
            improved = False
            for s1 in range(N_SLOTS):
                for s2 in range(s1 + 1, N_SLOTS):
                    for a in range(N_CORES):
                        for b2 in range(N_CORES):
                            cur = tot(slots)
                            slots[s1][a], slots[s2][b2] = (
                                slots[s2][b2],
                                slots[s1][a],
                            )
                            if tot(slots) < cur:
                                improved = True
                            else:
                                slots[s1][a], slots[s2][b2] = (
                                    slots[s2][b2],
                                    slots[s1][a],
                                )
            if best is None or tot(slots) < best[0]:
                best = (tot(slots), [list(g) for g in slots])
    slots = best[1]
    slots.sort(
        key=lambda g: _slot_cost(max(jt[b] for b in g), max(it[b] for b in g)),
        reverse=True,
    )
    return slots


def _build_program(slot_shapes):
    """slot_shapes: list of (J, I) per slot; one batch per core per slot."""
    import concourse.bacc as bacc
    import concourse.mybir as mybir
    import concourse.tile as tile

    f32 = mybir.dt.float32
    f16 = mybir.dt.float16
    Copy = mybir.ActivationFunctionType.Copy

    nc = bacc.Bacc("TRN2", debug=False)

    oqT_d, ocT_d, oqN_d, out_d, d_d = [], [], [], [], []
    for s in range(len(slot_shapes)):
        oqT_d.append(
            nc.declare_dram_parameter(f"oqT{s}", [KT, 128, QTW], f16, isOutput=False)
        )
        ocT_d.append(
            nc.declare_dram_parameter(f"ocT{s}", [KT, 128, CTW], f16, isOutput=False)
        )
        oqN_d.append(
            nc.declare_dram_parameter(f"oqN{s}", [Tq, H], f16, isOutput=False)
        )
        out_d.append(
            nc.declare_dram_parameter(f"out{s}", [Tc, H], f16, isOutput=True)
        )
        d_d.append(
            nc.declare_dram_parameter(f"dvec{s}", [1, Tc], f32, isOutput=True)
        )
    wt_d = nc.declare_dram_parameter("wt", [KT, 128, H], f16, isOutput=False)

    with tile.TileContext(nc) as tc:
        with (
            tc.tile_pool(name="const", bufs=1) as cpool,
            tc.tile_pool(name="inp", bufs=4) as ipool,
            tc.tile_pool(name="work", bufs=1) as wpool,
            tc.tile_pool(name="outp", bufs=4) as opool,
            tc.tile_pool(name="ps_u", bufs=2, space="PSUM") as ps_u,
            tc.tile_pool(name="ps_s", bufs=2, space="PSUM") as ps_s,
            tc.tile_pool(name="ps_c", bufs=3, space="PSUM") as ps_c,
            tc.tile_pool(name="ps_d", bufs=1, space="PSUM") as ps_d,
        ):
            ones_s = cpool.tile([1, 1], f32)
            nc.vector.memset(ones_s, 1.0)

            # PE warm-up: a short dependency-free matmul chain right after
            # boot ramps the PE out of its low pstate before the first real
            # operands land (~11us in); results are discarded. The dummy
            # PSUM group (dmisc bank) is free until slot 0's k-major Linear
            # reaches its 8th accumulator (~13us).
            dummy = cpool.tile([128, 512], f16, tag="dummy", name="dummy")
            nc.vector.memset(dummy, 0.0)
            dps_w = ps_d.tile([128, 512], f32, tag="dmisc", name="warm")
            for w in range(8):
                nc.tensor.matmul(
                    dps_w,
                    dummy[:, :128],
                    dummy[:, :512],
                    start=True,
                    stop=True,
                    skip_group_check=True,
                )

            wt_k = [cpool.tile([128, H], f16, tag=f"wt{k}", name=f"wt{k}") for k in range(KT)]

            def d_accum(e_tiles, dps, ic, s):
                # pre-add e pairs on DVE (16-bit 2x rate) so the PE's
                # partition-reduce runs over half as many rows
                J = len(e_tiles)
                groups = []
                k = 0
                while k + 1 < J:
                    ep = wpool.tile([128, ic], mybir.dt.float16, tag=f"ep{k//2}", name=f"ep{k//2}_{s}")
                    nc.vector.tensor_tensor(
                        out=ep, in0=e_tiles[k], in1=e_tiles[k + 1],
                        op=mybir.AluOpType.add,
                    )
                    groups.append(ep)
                    k += 2
                if k < J:
                    groups.append(e_tiles[k])
                for gi, g in enumerate(groups):
                    nc.tensor.matmul(
                        dps,
                        ones,
                        g,
                        start=(gi == 0),
                        stop=(gi == len(groups) - 1),
                        skip_group_check=True,
                    )

            def u_evict(o, dst, src):
                # alternate engines so the evict chain is ~2x faster; the
                # score matmuls consume u[o] in order
                if o % 2 == 0:
                    nc.vector.tensor_copy(out=dst, in_=src)
                else:
                    nc.scalar.activation(out=dst, in_=src, func=Copy)

            for s, (J, I) in enumerate(slot_shapes):
                qc, ic = J * 128, I * 128
                oqT_k = [ipool.tile([128, QTW], f16, tag=f"oqT{k}", name=f"oqT{k}_{s}") for k in range(KT)]
                ocT_k = [ipool.tile([128, CTW], f16, tag=f"ocT{k}", name=f"ocT{k}_{s}") for k in range(KT)]
                oqN = ipool.tile([128, JT, H], f16, tag="oqN", name=f"oqN_{s}")
                qb = oqT_k[KT - 1][:, Tq : Tq + J]
                ones = ocT_k[0][:, Tc : Tc + 1]
                for k in range(KT):
                    if s == 0:
                        nc.sync.dma_start(out=wt_k[k], in_=wt_d[k])
                    nc.sync.dma_start(out=oqT_k[k], in_=oqT_d[s][k])
                for k in range(KT):
                    nc.sync.dma_start(out=ocT_k[k], in_=ocT_d[s][k])
                for j in range(JT):
                    nc.sync.dma_start(
                        out=oqN[:, j, :], in_=oqN_d[s][j * 128 : (j + 1) * 128, :]
                    )

                # ---- Linear: u[o, j] = W@o_q.T  (the 1/32 scale rides in
                # the Exp activation's scale argument) ----
                u = wpool.tile([128, OT, qc], f16, tag="u", name=f"u_{s}")
                e_tiles = []
                dps = ps_d.tile([1, Tc], f32, tag="dmisc", name=f"dps_{s}")
                if s == 0:
                    ups_o = [
                        ps_u.tile([128, qc], f32, tag="ups", name="ups_a"),
                        ps_u.tile([128, qc], f32, tag="ups", name="ups_b"),
                        ps_s.tile([128, qc], f32, tag="sps", name="ups_c"),
                        ps_s.tile([128, qc], f32, tag="sps", name="ups_d"),
                        ps_c.tile([128, qc], f32, tag="cps", name="ups_e"),
                        ps_c.tile([128, qc], f32, tag="cps", name="ups_f"),
                        ps_c.tile([128, qc], f32, tag="cps", name="ups_g"),
                        ps_d.tile([128, qc], f32, tag="dmisc", name="ups_h"),
                    ]
                    for k in range(KT):
                        for o in range(OT):
                            nc.tensor.matmul(
                                ups_o[o],
                                wt_k[k][:, o * 128 : (o + 1) * 128],
                                oqT_k[k][:, :qc],
                                start=(k == 0),
                                stop=(k == KT - 1),
                            )
                    for o in range(OT):
                        u_evict(o, u[:, o, :], ups_o[o])

                    # ---- score o-outer: each o step needs only ocT_k[o],
                    # so the PE chases the ocT DMA tail ----
                    sps_j = [
                        ps_s.tile([128, ic], f32, tag="sps", name="sps_j0"),
                        ps_s.tile([128, ic], f32, tag="sps", name="sps_j1"),
                        ps_u.tile([128, ic], f32, tag="ups", name="sps_j2"),
                        ps_u.tile([128, ic], f32, tag="ups", name="sps_j3"),
                    ][:J]
                    for o in range(OT):
                        for jt_ in range(J):
                            nc.tensor.matmul(
                                sps_j[jt_],
                                u[:, o, jt_ * 128 : (jt_ + 1) * 128],
                                ocT_k[o][:, :ic],
                                start=(o == 0),
                                stop=(o == OT - 1),
                            )
                    for jt_ in range(J):
                        e = wpool.tile([128, ic], f16, tag=f"e{jt_}", name=f"e{jt_}_{s}")
                        nc.scalar.activation(
                            out=e,
                            in_=sps_j[jt_],
                            func=mybir.ActivationFunctionType.Exp,
                            bias=qb[:, jt_ : jt_ + 1],
                            scale=SCALE,
                        )
                        e_tiles.append(e)
                    d_accum(e_tiles, dps, ic, s)
                else:
                    for o in range(OT):
                        ups = ps_u.tile([128, qc], f32, tag="ups")
                        for k in range(KT):
                            nc.tensor.matmul(
                                ups,
                                wt_k[k][:, o * 128 : (o + 1) * 128],
                                oqT_k[k][:, :qc],
                                start=(k == 0),
                                stop=(k == KT - 1),
                            )
                        u_evict(o, u[:, o, :], ups)

                    # ---- score jt-outer + exp, denominator accumulation
                    # interleaved one step behind so its chain latency hides
                    for jt_ in range(J):
                        sps = ps_s.tile([128, ic], f32, tag="sps")
                        for o in range(OT):
                            nc.tensor.matmul(
                                sps,
                                u[:, o, jt_ * 128 : (jt_ + 1) * 128],
                                ocT_k[o][:, :ic],
                                start=(o == 0),
                                stop=(o == OT - 1),
                            )
                        e = wpool.tile([128, ic], f16, tag=f"e{jt_}", name=f"e{jt_}_{s}")
                        nc.scalar.activation(
                            out=e,
                            in_=sps,
                            func=mybir.ActivationFunctionType.Exp,
                            bias=qb[:, jt_ : jt_ + 1],
                            scale=SCALE,
                        )
                        e_tiles.append(e)
                    d_accum(e_tiles, dps, ic, s)

                osb_tiles = {}

                def ctx_group(itb, hb, J=J, s=s, e_tiles=e_tiles, oqN=oqN, osb_tiles=osb_tiles):
                    if itb not in osb_tiles:
                        osb_tiles[itb] = opool.tile(
                            [128, H], f16, tag="osb", name=f"osb{itb}_{s}"
                        )
                    cps = ps_c.tile([128, 512], f32, tag="cps", name=f"cps{itb}{hb}_{s}")
                    for jt_ in range(J):
                        nc.tensor.matmul(
                            cps,
                            e_tiles[jt_][:, itb * 128 : (itb + 1) * 128],
                            oqN[:, jt_, hb * 512 : (hb + 1) * 512],
                            start=(jt_ == 0),
                            stop=(jt_ == J - 1),
                        )
                    return cps

                def ctx_evict(itb, hb, cps, s=s, osb_tiles=osb_tiles, out=out_d[s]):
                    # unnormalized context; the host divides by d (shipped
                    # separately), so the evict is a plain copy and can
                    # alternate engines like u_evict does
                    osb = osb_tiles[itb]
                    if hb % 2 == 0:
                        nc.vector.tensor_copy(
                            out=osb[:, hb * 512 : (hb + 1) * 512], in_=cps
                        )
                    else:
                        nc.scalar.activation(
                            out=osb[:, hb * 512 : (hb + 1) * 512], in_=cps, func=Copy
                        )
                    nc.sync.dma_start(
                        out=out[
                            itb * 128 : (itb + 1) * 128, hb * 512 : (hb + 1) * 512
                        ],
                        in_=osb[:, hb * 512 : (hb + 1) * 512],
                    )

                # d ships to the host (the division happens there); the
                # copy+DMA drain while the first ctx groups run
                cps00 = ctx_group(0, 0)
                dsb = wpool.tile([1, Tc], f32, tag="dsb", name=f"dsb_{s}")
                nc.vector.tensor_copy(out=dsb, in_=dps)
                nc.sync.dma_start(out=d_d[s][:, :], in_=dsb)

                cps01 = ctx_group(0, 1)
                ctx_evict(0, 0, cps00)
                ctx_evict(0, 1, cps01)
                for itb in range(1, I):
                    for hb in range(HB):
                        cps = ctx_group(itb, hb)
                        ctx_evict(itb, hb, cps)

    nc.compile()
    return nc


def _host_inputs(o_c, o_q, W, q_lengths, slots):
    """Per-core input maps (host-side sharding + re-layout). All slabs are
    full-size (uniform DMA shapes across slots; see module docstring)."""
    NEG16 = np.float16(-60000.0)  # exp(x - 60000) == 0 exactly in fp32
    wt_host = np.ascontiguousarray(W.T.reshape(KT, 128, H).astype(np.float16))
    jidx = np.arange(JT)[None, :] * 128 + np.arange(128)[:, None]  # [128, JT]
    in_maps = [{"wt": wt_host} for _ in range(N_CORES)]
    for s, grp in enumerate(slots):
        for c, g in enumerate(grp):
            oqT = np.zeros((KT, 128, QTW), np.float16)
            oqT[:, :, :Tq] = o_q[g].T.reshape(KT, 128, Tq).astype(np.float16)
            ql = int(q_lengths[g])
            # qb (exp bias: 0 valid / -60000 masked) rides in the last slab
            oqT[KT - 1, :, Tq : Tq + JT] = np.where(
                jidx < ql, np.float16(0.0), NEG16
            )
            ocT = np.zeros((KT, 128, CTW), np.float16)
            ocT[:, :, :Tc] = o_c[g].T.reshape(KT, 128, Tc).astype(np.float16)
            ocT[0, :, Tc] = 1.0  # ones column for the denominator matmul
            in_maps[c][f"oqT{s}"] = oqT
            in_maps[c][f"ocT{s}"] = ocT
            in_maps[c][f"oqN{s}"] = np.ascontiguousarray(o_q[g].astype(np.float16))
    return in_maps


def kernel(**inputs) -> np.ndarray:
    o_c = np.asarray(inputs["o_c"], dtype=np.float32)
    o_q = np.asarray(inputs["o_q"], dtype=np.float32)
    W = np.asarray(inputs["W"], dtype=np.float32)
    q_lengths = np.asarray(inputs["q_lengths"]).astype(np.int64)
    c_lengths = np.asarray(inputs["c_lengths"]).astype(np.int64)

    from concourse.bass_utils import run_bass_kernel_spmd

    jt = [min(max(-(-int(q) // 128), 1), JT) for q in q_lengths]
    it = [min(max(-(-int(cl) // 128), 1), IT) for cl in c_lengths]
    slots = _assign_slots(jt, it)
    slot_shapes = [
        (max(jt[g] for g in grp), max(it[g] for g in grp)) for grp in slots
    ]

    in_maps = _host_inputs(o_c, o_q, W, q_lengths, slots)
    nc = _build_program(slot_shapes)

    trace = bool(int(os.environ.get("KERNEL_TRACE", "0")))
    res = run_bass_kernel_spmd(
        nc, in_maps, core_ids=list(range(N_CORES)), trace=trace
    )
    if trace:
        kernel.last_results = res

    out = np.zeros((B, Tc, H), dtype=np.float32)
    for s, grp in enumerate(slots):
        for c, g in enumerate(grp):
            dev = res.results[c][f"out{s}"]
            dvec = res.results[c][f"dvec{s}"].reshape(-1)
            cl = int(c_lengths[g])
            out[g, :cl] = dev[:cl].astype(np.float32) / dvec[:cl, None]
    return out
